# revision 36
# baseline (speedup 1.0000x reference)
"""Trainium2 Bass kernel for GQA attention block (nn_Attention_81372450390110).

Module: y = AttnOut(x) with q/k RMSNorm + interleaved RoPE + causal GQA
(NH=16 q heads, KVH=4 kv heads, HD=128, D=2048, B=2, S=2048).

Sharding: 8 cores = 2 batches x 4 KV groups. Core c handles batch c//4 and
KV group c%4 (4 q heads + 1 kv head). Each core computes a full [S, D]
partial of the output projection (row-parallel over heads); the host sums
the 4 group-partials per batch.

Layout strategy (feature-major activations):
  - host passes xT = x[b].T so the D contraction dim lands on partitions
  - qT/kT computed as [HD, S] (lhsT = weight chunk); v computed token-major
    directly (lhsT = x block, rhs = wv chunk) -> no PE transposes for v
  - rmsnorm: sum-of-squares broadcast with ONE all-ones matmul, then ACT
    Sqrt + DVE reciprocal. ACT functions are kept to {Square, Sqrt, Copy}
    in phase A and {Exp, Copy} in phase B: each set lives in one HW
    activation table (sqrt_and_others / exp_and_others), so there are only
    2 ACT_TABLE_LOADs in the whole kernel (a Ln-based variant thrashed 41
    table loads = 53us because Ln lives in its own table set)
  - rope pairing interleaved (partner = adjacent partition): partner swap
    is one DVE stream_shuffle (mask i^1); rope muls in bf16 on GPSIMD
    (SBUF-only engine, otherwise idle); DVE ops avoid f32 outputs where
    possible (f32 tensor_tensor measured ~2x slower than bf16-out)
  - scores transposed: sT[k, q] = kT_blk.T @ qT_blk; softmax without max
    subtraction (rmsnorm bounds |scores| <= sqrt(HD)); exp processed in
    PAIRS of k-blocks from a 2-bank [128,1024] PSUM tile to halve the
    ~210ns per-instruction ACT overhead; causal tri-mask muls on GPSIMD
  - P@V q-major (lhsT = P block, rhs = v block + ones column) -> the
    softmax denominator falls out as the 129th column; normalize on DVE
    (tensor_scalar) then PE-transpose into attT for the o-projection;
    transpose PSUM target shares the att tag (bitcast view) to fit the
    8-bank PSUM budget
  - o-projection emitted in (token-stripe, d-block) units interleaved into
    the NEXT qt-group's kb loop so the PE always has matmul work while ACT
    computes exp; y casts alternate ACT/DVE into a [128, 2048] stripe
    buffer, one DMA per stripe
  - startup: PE warmup (no-dep matmuls) covers the initial DMA stream;
    inputs arrive in need order (wq per head, xt in 8 pieces) on two
    queues so the first projection can start ~8us in; phase A's last
    norm chain is emitted before the v-projection so the phase B PSUM
    reuse never stalls the PE (stalls trigger HAM half-clock for ~10us)
"""

import os
import sys

sys.path.insert(0, "/opt/trn_rl_repo")

import numpy as np
import ml_dtypes

BF16 = ml_dtypes.bfloat16

B = 2
S = 2048
D = 2048
NH = 16
KVH = 4
HD = 128
THETA = 10000.0
EPS = 1e-6
NHL = NH // KVH  # q heads per core (4)
SCALE = 1.0 / float(np.sqrt(HD))
WARMUP = 32

_CACHED = {}


def build_nc(s=S, d=D, nhl=NHL, hd=HD):
    import concourse.mybir as mybir
    import concourse.tile as tile
    from concourse import bacc
    from concourse.masks import make_identity

    f32 = mybir.dt.float32
    f16 = mybir.dt.float16
    bf16 = mybir.dt.bfloat16
    AF = mybir.ActivationFunctionType

    kc_n = d // 128          # contraction chunks for projections
    nb_n = s // 512          # 512-token blocks
    qt_n = s // 512          # q tiles (512 wide) in attention
    kb_n = s // 128          # k blocks (128 wide)
    db_n = d // 512          # o-proj output chunks

    SHUF = [i ^ 1 for i in range(32)]  # rope partner swap (adjacent pairs)

    nc = bacc.Bacc("TRN2", target_bir_lowering=False, debug=False)

    xT_d = nc.dram_tensor("xT", (d, s), bf16, kind="ExternalInput")
    # weights arrive pre-shuffled by the host into the exact SBUF layout
    # ([partition, kc, m] flattened) so every DMA moves contiguous >=2KB
    # rows; the natural (kc p) m rearrange produced 256B descriptors and a
    # 9us wv transfer
    kc_n_ = d // 128
    wq_d = nc.dram_tensor("wq", (128, kc_n_ * nhl * hd), bf16,
                          kind="ExternalInput")
    wk_d = nc.dram_tensor("wk", (128, kc_n_ * hd), bf16,
                          kind="ExternalInput")
    wv_d = nc.dram_tensor("wv", (128, kc_n_ * hd), bf16,
                          kind="ExternalInput")
    wo_d = nc.dram_tensor("wo", (128, nhl * d), bf16, kind="ExternalInput")
    m1q_d = nc.dram_tensor("m1q", (hd, s), bf16, kind="ExternalInput")
    m2q_d = nc.dram_tensor("m2q", (hd, s), bf16, kind="ExternalInput")
    m1k_d = nc.dram_tensor("m1k", (hd, s), bf16, kind="ExternalInput")
    m2k_d = nc.dram_tensor("m2k", (hd, s), bf16, kind="ExternalInput")
    tri_d = nc.dram_tensor("tri", (128, 128), bf16, kind="ExternalInput")
    y_d = nc.dram_tensor("y", (s, d), f16, kind="ExternalOutput")

    with tile.TileContext(nc) as tc, nc.allow_low_precision(
        reason="bf16 compute by design; fp32 accumulation in PSUM"
    ):
        with (
            tc.tile_pool(name="const", bufs=1) as const,
            tc.tile_pool(name="persist", bufs=1) as persist,
        ):
            # ---- tiny consts first (cheap engine work, no DMA deps) ------
            warm = const.tile([128, 512], bf16, tag="warm")
            nc.vector.memset(warm[:], 0.0)
            ones128 = const.tile([128, 128], bf16, tag="ones128")
            nc.vector.memset(ones128[:], 1.0)
            ident = const.tile([128, 128], bf16, tag="ident")
            make_identity(nc, ident[:])
            tri_sb = const.tile([128, 128], bf16, tag="tri")
            eps_sb = const.tile([128, 1], f32, tag="eps")
            nc.vector.memset(eps_sb[:], EPS)
            dummy = const.tile([1, 1], bf16, tag="dummy")

            # ---- resident weights / coefficients -------------------------
            wq_sb = persist.tile([128, kc_n, nhl * hd], bf16, tag="wq")
            wq_re = wq_d.rearrange("p (kc m) -> p kc m", kc=kc_n)
            wk_sb = persist.tile([128, kc_n, hd], bf16, tag="wk")
            wk_re = wk_d.rearrange("p (kc m) -> p kc m", kc=kc_n)
            wv_sb = persist.tile([128, kc_n, hd], bf16, tag="wv")
            wv_re = wv_d.rearrange("p (kc m) -> p kc m", kc=kc_n)
            wo_sb = persist.tile([128, nhl, d], bf16, tag="wo")
            wo_re = wo_d.rearrange("p (h m) -> p h m", h=nhl)

            m1q_sb = persist.tile([hd, s], bf16, tag="m1q")
            m2q_sb = persist.tile([hd, s], bf16, tag="m2q")
            m1k_sb = persist.tile([hd, s], bf16, tag="m1k")
            m2k_sb = persist.tile([hd, s], bf16, tag="m2k")

            # ---- persistent activations ---------------------------------
            qT_sb = [persist.tile([hd, s], bf16, tag=f"qT{h}", name=f"qT{h}")
                     for h in range(nhl)]
            kT_sb = persist.tile([hd, s], bf16, tag="kT")
            v_sb = persist.tile([128, kb_n, hd + 1], bf16, tag="v")
            nc.vector.memset(v_sb[:, :, hd:hd + 1], 1.0)
            attT_sb = [persist.tile([hd, s], bf16, tag=f"attT{h}",
                                    name=f"attT{h}") for h in range(nhl)]
            # last token block staged persistently: part of its
            # v-projection runs in phase B (as qt0 PE filler), so it must
            # not alias phase B work tiles
            xt3 = persist.tile([128, kc_n, 512], bf16, tag="xt3")

            xT_re = xT_d.rearrange("(kc p) n -> p kc n", p=128)

            # ================= Phase A: projections + norm + rope =========
            with (
                tc.tile_pool(name="xtp", bufs=2) as xtp,
                tc.tile_pool(name="workA", bufs=3) as wa,
                tc.tile_pool(name="psA", bufs=2, space="PSUM") as psA,
            ):
                # PE warmup: dummy matmuls with no DMA deps, so the HAM
                # clock reaches 8/8 and the p-state ramps while the first
                # input DMAs are still in flight.
                wps = psA.tile([128, 512], f32, tag="q_ps", bufs=5,
                               name="wps")
                for _ in range(WARMUP):
                    nc.tensor.matmul(wps[:], warm[:, 0:128], warm[:])

                # ---- input DMAs: need-ordered, two queues ----------------
                # nb0 is consumed kc-ordered (see below), so weights stream
                # kc-major: wk whole (small, needed for every kc group),
                # then wq in 2-kc pieces matching the xt piece arrivals.
                xts = [xtp.tile([128, kc_n, 512], bf16, tag="xt",
                                name="xt0")]
                for j in range(6):   # first block in pieces: matmuls can
                    nc.sync.dma_start(   # start as soon as piece 0 lands
                        xts[0][:, 2 * j:2 * j + 2, :],
                        xT_re[:, 2 * j:2 * j + 2, 0:512])
                nc.gpsimd.dma_start(wk_sb[:], wk_re[:])
                for j in range(8):
                    nc.gpsimd.dma_start(wq_sb[:, 2 * j:2 * j + 2, :],
                                        wq_re[:, 2 * j:2 * j + 2, :])
                nc.gpsimd.dma_start(wv_sb[:], wv_re[:])
                nc.gpsimd.dma_start(m1q_sb[:], m1q_d[:, :])
                nc.gpsimd.dma_start(m2q_sb[:], m2q_d[:, :])
                nc.gpsimd.dma_start(m1k_sb[:], m1k_d[:, :])
                nc.gpsimd.dma_start(m2k_sb[:], m2k_d[:, :])

                def chain_sq(q_ps):
                    sq = wa.tile([128, 512], bf16, tag="sq", name="sq")
                    nc.scalar.activation(sq[:], q_ps[:], AF.Square)
                    return sq

                def chain_rest(sq, q_ps, t, cs):
                    # rmsnorm: broadcast sum-of-squares via one all-ones
                    # matmul; rsqrt = ACT Sqrt + DVE reciprocal (all ACT
                    # funcs stay in the sqrt_and_others table). rope:
                    # partner swap via stream_shuffle, coefficient muls in
                    # bf16 on GPSIMD.
                    ssb = psA.tile([128, 512], f32, tag="ssq", bufs=1,
                                   name="ssb")
                    nc.tensor.matmul(ssb[:], ones128[:], sq[:])
                    srt = wa.tile([128, 512], f32, tag="srt", name="srt")
                    nc.scalar.activation(srt[:], ssb[:], AF.Sqrt,
                                         scale=1.0 / hd, bias=eps_sb[:])
                    rb = wa.tile([128, 512], f32, tag="rb", name="rb")
                    nc.vector.reciprocal_approx_fast(rb[:], srt[:])
                    qn = wa.tile([128, 512], bf16, tag="qn", name="qn")
                    nc.vector.tensor_mul(qn[:], q_ps[:], rb[:])
                    qsw = wa.tile([128, 512], bf16, tag="qsw", name="qsw")
                    nc.vector.stream_shuffle(qsw[:], qn[:], SHUF)
                    m1 = m1q_sb if t < nhl else m1k_sb
                    m2 = m2q_sb if t < nhl else m2k_sb
                    t1 = wa.tile([128, 512], bf16, tag="t1", name="t1")
                    nc.gpsimd.tensor_mul(t1[:], qn[:], m1[:, cs])
                    t2 = wa.tile([128, 512], bf16, tag="t2", name="t2")
                    nc.gpsimd.tensor_mul(t2[:], qsw[:], m2[:, cs])
                    dest = qT_sb[t] if t < nhl else kT_sb
                    nc.gpsimd.tensor_add(dest[:, cs], t1[:], t2[:])

                def norm_rope_chain(q_ps, t, cs):
                    chain_rest(chain_sq(q_ps), q_ps, t, cs)

                pending = None
                for nb in range(nb_n):
                    cs = slice(nb * 512, (nb + 1) * 512)
                    xt = xts[nb]
                    if nb + 1 < nb_n:  # prefetch next block, 1 instruction
                        if nb + 1 == nb_n - 1:
                            nxt = xt3
                        else:
                            nxt = xtp.tile([128, kc_n, 512], bf16,
                                           tag="xt", name=f"xt{nb + 1}")
                        nc.sync.dma_start(
                            nxt[:],
                            xT_re[:, :, (nb + 1) * 512:(nb + 2) * 512])
                        xts.append(nxt)
                    if nb == 0:
                        # last two xt0 pieces AFTER the xt1 prefetch: they
                        # feed the staggered tail, which runs latest
                        for j in range(6, 8):
                            nc.sync.dma_start(
                                xts[0][:, 2 * j:2 * j + 2, :],
                                xT_re[:, 2 * j:2 * j + 2, 0:512])
                    if nb == 1:   # inputs not needed until phase B: keep
                        nc.sync.dma_start(tri_sb[:], tri_d[:, :])
                    if nb == 2:   # early DMA bandwidth for xt prefetches
                        nc.sync.dma_start(wo_sb[:], wo_re[:])

                    if nb == 0:
                        # kc-ordered for the first 12 chunks: 5 concurrent
                        # PSUM accumulators, so each arriving (xt, wq)
                        # piece immediately yields matmuls and the PE
                        # tracks the DMA stream. The last 4 chunks run
                        # t-major so the 5 targets FINISH staggered and
                        # their norm chains don't pile up on ACT at once.
                        q_pss = [psA.tile([128, 512], f32, tag="q_ps",
                                          bufs=5, name=f"q0_{t}")
                                 for t in range(nhl + 1)]
                        for kc in range(kc_n - 4):
                            for t in range(nhl + 1):
                                lhsT = (wq_sb[:, kc, t * hd:(t + 1) * hd]
                                        if t < nhl else wk_sb[:, kc, :])
                                nc.tensor.matmul(
                                    q_pss[t][:], lhsT, xt[:, kc, :],
                                    start=(kc == 0), stop=False,
                                )
                        for t in range(nhl + 1):
                            for kc in range(kc_n - 4, kc_n):
                                lhsT = (wq_sb[:, kc, t * hd:(t + 1) * hd]
                                        if t < nhl else wk_sb[:, kc, :])
                                nc.tensor.matmul(
                                    q_pss[t][:], lhsT, xt[:, kc, :],
                                    start=False, stop=(kc == kc_n - 1),
                                )
                            if pending is not None:
                                norm_rope_chain(*pending)
                            pending = (q_pss[t], t, cs)
                    else:
                        # q heads then k (k FIRST in the last block, so
                        # phase B's first S matmul never waits on the k
                        # chain): projection MMs now, chain deferred one
                        # tensor so its ACT/DVE/GPS work overlaps the next
                        # target's projection matmuls
                        t_order = ([nhl] + list(range(nhl))
                                   if nb == nb_n - 1 else
                                   list(range(nhl + 1)))
                        tail2 = []
                        for idx, t in enumerate(t_order):
                            q_ps = psA.tile([128, 512], f32, tag="q_ps",
                                            bufs=5)
                            for kc in range(kc_n):
                                if t < nhl:
                                    lhsT = wq_sb[:, kc, t * hd:(t + 1) * hd]
                                else:
                                    lhsT = wk_sb[:, kc, :]
                                nc.tensor.matmul(
                                    q_ps[:], lhsT, xt[:, kc, :],
                                    start=(kc == 0), stop=(kc == kc_n - 1),
                                )
                            if nb == nb_n - 1 and idx >= 3:
                                # last block's last two chains are SPLIT:
                                # ACT Square now, the rest emitted between
                                # v-proj fills below -- the in-order PE
                                # queue must never reach a chain matmul
                                # before its ACT input is ready
                                if pending is not None:
                                    norm_rope_chain(*pending)
                                    pending = None
                                tail2.append((chain_sq(q_ps), q_ps, t))
                            else:
                                if pending is not None:
                                    norm_rope_chain(*pending)
                                pending = (q_ps, t, cs)

                    if nb == nb_n - 1:
                        # interleave the last block's v-projection (pure PE
                        # work) with the split chain remainders, so the
                        # chains' ACT/DVE/GPS latency is fully hidden and
                        # phase B starts with all PSUM banks ready
                        def vproj3(tb):
                            vp = psA.tile([128, 512], f32, tag="vps",
                                          bufs=2, name="v_ps3")
                            for kc in range(kc_n):
                                nc.tensor.matmul(
                                    vp[:, 0:hd],
                                    xt[:, kc, tb * 128:(tb + 1) * 128],
                                    wv_sb[:, kc, :],
                                    start=(kc == 0), stop=(kc == kc_n - 1),
                                )
                            nc.scalar.copy(
                                v_sb[:, 12 + tb, 0:hd], vp[:, 0:hd])
                        vproj3(0)
                        chain_rest(*tail2[0], cs)
                        vproj3(1)
                        chain_rest(*tail2[1], cs)
                        vproj3(2)
                        # boundary bridge: no-dep matmuls keep the PE busy
                        # (and HAM at 8/8) while the last chains' DVE work
                        # frees the PSUM banks phase B reuses
                        brg = psA.tile([128, 512], f32, tag="vps", bufs=2,
                                       name="bridge")
                        for _ in range(8):
                            nc.tensor.matmul(brg[:], warm[:, 0:128],
                                             warm[:])
                        continue

                    # v: token-major directly (lhsT = x block, rhs = wv
                    # chunk) -> no transposes
                    v_ps = psA.tile([128, 512], f32, tag="vps", bufs=2,
                                    name="v_ps")
                    for tb in range(4):
                        for kc in range(kc_n):
                            nc.tensor.matmul(
                                v_ps[:, tb * 128:(tb + 1) * 128],
                                xt[:, kc, tb * 128:(tb + 1) * 128],
                                wv_sb[:, kc, :],
                                start=(kc == 0), stop=(kc == kc_n - 1),
                            )
                    for tb in range(4):
                        nc.vector.tensor_copy(
                            v_sb[:, nb * 4 + tb, 0:hd],
                            v_ps[:, tb * 128:(tb + 1) * 128])
                if pending is not None:
                    norm_rope_chain(*pending)
                    pending = None

            # ================= Phase B: causal flash attention ============
            with (
                tc.tile_pool(name="workB", bufs=4) as wb,
                tc.tile_pool(name="psB", bufs=2, space="PSUM") as psB,
            ):
                backlog = []   # (tt, db) o-proj units, drained in kb loops
                ybig = {}

                # preload the Exp ACT table during phase A's tail (the
                # table load binds to the first Exp in ACT queue order);
                # the dummy tile lives in the const pool so the exp never
                # waits on recycled work-tile memory
                nc.scalar.activation(dummy[:], eps_sb[0:1, 0:1], AF.Exp)

                vleft = [3]   # last nb3 v-proj block: qt0 PE filler

                def vproj_b(tb):
                    vp = psB.tile([128, hd + 1], f32, tag="att", bufs=4,
                                  name="vps_b")
                    for kc in range(kc_n):
                        nc.tensor.matmul(
                            vp[:, 0:hd],
                            xt3[:, kc, tb * 128:(tb + 1) * 128],
                            wv_sb[:, kc, :],
                            start=(kc == 0), stop=(kc == kc_n - 1),
                        )
                    nc.vector.tensor_copy(v_sb[:, 12 + tb, 0:hd],
                                          vp[:, 0:hd])

                def oproj_unit(tt, db, act_cast=False):
                    if db == 0:
                        ybig[tt] = wb.tile([128, d], f16, tag="ybig",
                                           bufs=2, name=f"ybig{tt}")
                    y_ps = psB.tile([128, 1024], f32, tag="spair", bufs=2,
                                    name="y_ps")
                    for hh in range(nhl):
                        nc.tensor.matmul(
                            y_ps[:, 0:512],
                            attT_sb[hh][:, tt * 128:(tt + 1) * 128],
                            wo_sb[:, hh, db * 512:(db + 1) * 512],
                            start=(hh == 0), stop=(hh == nhl - 1),
                        )
                    dst = ybig[tt][:, db * 512:(db + 1) * 512]
                    if act_cast:
                        nc.scalar.copy(dst, y_ps[:, 0:512])
                    else:
                        nc.vector.tensor_copy(dst, y_ps[:, 0:512])
                    if db == db_n - 1:
                        nc.sync.dma_start(
                            y_d[tt * 128:(tt + 1) * 128, :], ybig[tt][:])
                        del ybig[tt]

                prev_tp = None   # previous head's normalized stripes:
                                 # its transposes run under the NEXT
                                 # head's S matmuls (the per-head tail is
                                 # inherently serial otherwise)

                def flush_tp():
                    ph, pqt, patt_n = prev_tp
                    for qs in range(4):
                        # transpose target shares the att tag (bitcast
                        # bf16 view) to stay within 8 PSUM banks
                        tpt = psB.tile([128, hd + 1], f32, tag="att",
                                       bufs=4, name="tp")
                        tp = tpt[:, 0:64].bitcast(bf16)
                        nc.tensor.transpose(tp, patt_n[qs][:], ident[:])
                        nc.vector.tensor_copy(
                            attT_sb[ph][:, pqt * 512 + qs * 128:
                                        pqt * 512 + (qs + 1) * 128],
                            tp,
                        )

                for qt in range(qt_n):
                    # spread the available o-proj units evenly over this
                    # qt group's pairs (clustered draining leaves the
                    # remaining pairs ACT-bound by ~200ns each)
                    qt_pairs = nhl * (2 * qt + 2)
                    qt_backlog0 = len(backlog)
                    qt_pc = 0
                    qt_drained = 0
                    for h in range(nhl):
                        nkb = 4 * qt + 4
                        np_ = nkb // 2   # k-block pairs
                        att_n = {}
                        s_tiles = {}

                        def emit_pair(j):
                            # two k-blocks' scores into one 2-bank PSUM
                            # tile (each matmul stays within one bank).
                            # Diagonal blocks get the causal mask ADDED in
                            # PSUM (-6e4 above the diagonal, via mask.T @
                            # identity): exp then yields exact zeros, so
                            # no per-block mask multiply serializes the
                            # exp->PV chain
                            sp = psB.tile([128, 1024], f32, tag="spair",
                                          name="s_ps", bufs=2)
                            for i in range(2):
                                kb = 2 * j + i
                                r = kb - 4 * qt
                                c0 = 128 * r if r > 0 else 0
                                nc.tensor.matmul(
                                    sp[:, 512 * i + c0:512 * (i + 1)],
                                    kT_sb[:, kb * 128:(kb + 1) * 128],
                                    qT_sb[h][:,
                                             qt * 512 + c0:(qt + 1) * 512],
                                )
                                if r >= 0:
                                    nc.tensor.matmul(
                                        sp[:, 512 * i + c0:
                                           512 * i + c0 + 128],
                                        tri_sb[:], ident[:],
                                        start=False, stop=True,
                                        skip_group_check=True,
                                    )
                            s_tiles[j] = sp

                        emit_pair(0)
                        if np_ > 1:
                            emit_pair(1)
                        if prev_tp is not None:
                            flush_tp()
                        att = [psB.tile([128, hd + 1], f32, tag="att",
                                        bufs=4, name=f"att{i}")
                               for i in range(4)]
                        for j in range(np_):
                            if j + 2 < np_:
                                emit_pair(j + 2)
                            # PE filler while ACT computes exp: o-proj
                            # units (proportionally spread), or the
                            # deferred v-projection during qt=0
                            qt_pc += 1
                            target = (qt_pc * qt_backlog0 +
                                      qt_pairs - 1) // qt_pairs
                            while backlog and qt_drained < target:
                                oproj_unit(*backlog.pop(0))
                                qt_drained += 1
                            if qt == 0 and vleft and j == 0:
                                vproj_b(vleft.pop(0))
                            sp = s_tiles.pop(j)
                            p = wb.tile([128, 1024], bf16, tag="p", bufs=4)
                            kb0 = 2 * j
                            r0 = kb0 - 4 * qt
                            if r0 + 1 < 0:
                                nc.scalar.activation(p[:], sp[:], AF.Exp,
                                                     scale=SCALE)
                            else:
                                # diagonal pair: exp the two valid regions,
                                # tri-mask the diagonal blocks on GPSIMD
                                c00 = 128 * r0 if r0 > 0 else 0
                                c01 = 128 * (r0 + 1)
                                nc.scalar.activation(
                                    p[:, c00:512], sp[:, c00:512],
                                    AF.Exp, scale=SCALE)
                                nc.scalar.activation(
                                    p[:, 512 + c01:1024],
                                    sp[:, 512 + c01:1024],
                                    AF.Exp, scale=SCALE)
                            for i in range(2):
                                kb = 2 * j + i
                                off = 512 * i
                                for qs in range(4):
                                    kmax = 4 * qt + qs
                                    if kb > kmax:
                                        continue
                                    nc.tensor.matmul(
                                        att[qs][:],
                                        p[:, off + qs * 128:
                                          off + (qs + 1) * 128],
                                        v_sb[:, kb, :],
                                        start=(kb == 0), stop=(kb == kmax),
                                    )
                                # the DVE normalize of a finished q stripe
                                # starts NOW, overlapping the remaining
                                # pairs; only the PE transposes stay in
                                # the head's tail
                                r = kb - 4 * qt
                                if r >= 0:
                                    rec = wb.tile([128, 1], f32, tag="rec",
                                                  bufs=4, name="rec")
                                    nc.vector.reciprocal(
                                        rec[:], att[r][:, hd:hd + 1])
                                    att_n[r] = wb.tile([128, 128], bf16,
                                                       tag="attn", bufs=4,
                                                       name=f"attn{r}")
                                    nc.vector.tensor_scalar_mul(
                                        att_n[r][:], att[r][:, 0:hd],
                                        rec[:])
                        prev_tp = (h, qt, att_n)
                    for tt in range(qt * 4, qt * 4 + 4):
                        for db in range(db_n):
                            backlog.append((tt, db))
                if prev_tp is not None:
                    flush_tp()
                    prev_tp = None
                n_tail = 0
                while backlog:
                    oproj_unit(*backlog.pop(0), act_cast=bool(n_tail % 2))
                    n_tail += 1

    nc.compile()
    return nc


def _rope_coeffs(norm_w, s=S, hd=HD):
    """Coefficient tiles [hd, s]: interleaved rope pairing (partner =
    adjacent lane), norm weight folded in.
      dest[2i]   = qn[2i]*cos_i*w[2i]   + qn[2i+1]*(-sin_i*w[2i])
      dest[2i+1] = qn[2i+1]*cos_i*w[2i+1] + qn[2i]*( sin_i*w[2i+1])
    with qs = shuffle(qn, pair swap), dest = qn*m1 + qs*m2."""
    w = np.asarray(norm_w, np.float64)
    pos = np.arange(s, dtype=np.float64)
    inv_freq = 1.0 / (THETA ** (np.arange(0, hd, 2, dtype=np.float64) / hd))
    ang = pos[None, :] * inv_freq[:, None]          # [half, s]
    cos, sin = np.cos(ang), np.sin(ang)
    m1 = np.empty((hd, s), np.float32)
    m2 = np.empty((hd, s), np.float32)
    m1[0::2] = cos * w[0::2, None]
    m1[1::2] = cos * w[1::2, None]
    m2[0::2] = -sin * w[0::2, None]
    m2[1::2] = sin * w[1::2, None]
    return m1, m2


def _host_prep(x, wq, wk, wv, wo, q_norm_w, k_norm_w):
    m1q, m2q = _rope_coeffs(q_norm_w)
    m1k, m2k = _rope_coeffs(k_norm_w)
    # additive causal mask, transposed for lhsT (mask.T @ I = mask):
    # M[k, q] = -6e4 where k > q; lhsT = M.T
    m = np.where(np.arange(128)[:, None] > np.arange(128)[None, :],
                 np.float32(-60000.0), np.float32(0.0))
    tri = np.ascontiguousarray(m.T).astype(BF16)

    def shuf_w(w):
        # (kc*128, m) -> (128, kc*m): row p holds [kc, m] in SBUF order
        kc = w.shape[0] // 128
        return np.ascontiguousarray(
            w.reshape(kc, 128, w.shape[1]).transpose(1, 0, 2)
            .reshape(128, kc * w.shape[1]))

    in_maps = []
    for c in range(8):
        b, g = c // 4, c % 4
        in_maps.append({
            "xT": np.ascontiguousarray(x[b].T).astype(BF16),
            "wq": shuf_w(
                wq[:, NHL * g * HD:NHL * (g + 1) * HD]).astype(BF16),
            "wk": shuf_w(wk[:, g * HD:(g + 1) * HD]).astype(BF16),
            "wv": shuf_w(wv[:, g * HD:(g + 1) * HD]).astype(BF16),
            "wo": shuf_w(
                wo[NHL * g * HD:NHL * (g + 1) * HD, :]).astype(BF16),
            "m1q": m1q.astype(BF16), "m2q": m2q.astype(BF16),
            "m1k": m1k.astype(BF16), "m2k": m2k.astype(BF16),
            "tri": tri,
        })
    return in_maps


def _install_ntff_shim():
    import types
    if "antenv.axon_hooks" in sys.modules:
        return
    mod = types.ModuleType("antenv.axon_hooks")
    _hook = [None]
    mod.set_axon_ntff_profile_hook = lambda h: _hook.__setitem__(0, h)
    mod.get_axon_ntff_profile_hook = lambda: _hook[0]
    sys.modules["antenv.axon_hooks"] = mod
    try:
        from trn_agent_boot.trn_boot import _ntff_profile_via_ctypes
        mod.set_axon_ntff_profile_hook(
            _ntff_profile_via_ctypes("/opt/axon/libaxon_pjrt.so")
        )
    except Exception:
        pass


LAST_EXEC_NS = None


def kernel(x, wq, wk, wv, wo, q_norm_w, k_norm_w):
    global LAST_EXEC_NS
    from concourse import bass_utils

    x = np.asarray(x)
    if "nc" not in _CACHED:
        _CACHED["nc"] = build_nc()
    nc = _CACHED["nc"]

    in_maps = _host_prep(
        np.asarray(x, np.float32), np.asarray(wq, np.float32),
        np.asarray(wk, np.float32), np.asarray(wv, np.float32),
        np.asarray(wo, np.float32), np.asarray(q_norm_w, np.float32),
        np.asarray(k_norm_w, np.float32),
    )
    trace = bool(int(os.environ.get("BASS_KERNEL_TRACE", "0")))
    if trace:
        _install_ntff_shim()
    res = bass_utils.run_bass_kernel_spmd(
        nc, in_maps, core_ids=list(range(8)), trace=trace
    )
    LAST_EXEC_NS = res.exec_time_ns
    y = np.zeros((B, S, D), np.float32)
    for c in range(8):
        y[c // 4] += res.results[c]["y"]
    return y


# revision 37
# speedup vs baseline: 1.0292x; 1.0292x over previous
"""Trainium2 Bass kernel for GQA attention block (nn_Attention_81372450390110).

Module: y = AttnOut(x) with q/k RMSNorm + interleaved RoPE + causal GQA
(NH=16 q heads, KVH=4 kv heads, HD=128, D=2048, B=2, S=2048).

Sharding: 8 cores = 2 batches x 4 KV groups. Core c handles batch c//4 and
KV group c%4 (4 q heads + 1 kv head). Each core computes a full [S, D]
partial of the output projection (row-parallel over heads); the host sums
the 4 group-partials per batch.

Layout strategy (feature-major activations):
  - host passes xT = x[b].T so the D contraction dim lands on partitions
  - qT/kT computed as [HD, S] (lhsT = weight chunk); v computed token-major
    directly (lhsT = x block, rhs = wv chunk) -> no PE transposes for v
  - rmsnorm: sum-of-squares broadcast with ONE all-ones matmul, then ACT
    Sqrt + DVE reciprocal. ACT functions are kept to {Square, Sqrt, Copy}
    in phase A and {Exp, Copy} in phase B: each set lives in one HW
    activation table (sqrt_and_others / exp_and_others), so there are only
    2 ACT_TABLE_LOADs in the whole kernel (a Ln-based variant thrashed 41
    table loads = 53us because Ln lives in its own table set)
  - rope pairing interleaved (partner = adjacent partition): partner swap
    is one DVE stream_shuffle (mask i^1); rope muls in bf16 on GPSIMD
    (SBUF-only engine, otherwise idle); DVE ops avoid f32 outputs where
    possible (f32 tensor_tensor measured ~2x slower than bf16-out)
  - scores transposed: sT[k, q] = kT_blk.T @ qT_blk; softmax without max
    subtraction (rmsnorm bounds |scores| <= sqrt(HD)); exp processed in
    PAIRS of k-blocks from a 2-bank [128,1024] PSUM tile to halve the
    ~210ns per-instruction ACT overhead; causal tri-mask muls on GPSIMD
  - P@V q-major (lhsT = P block, rhs = v block + ones column) -> the
    softmax denominator falls out as the 129th column; normalize on DVE
    (tensor_scalar) then PE-transpose into attT for the o-projection;
    transpose PSUM target shares the att tag (bitcast view) to fit the
    8-bank PSUM budget
  - o-projection emitted in (token-stripe, d-block) units interleaved into
    the NEXT qt-group's kb loop so the PE always has matmul work while ACT
    computes exp; y casts alternate ACT/DVE into a [128, 2048] stripe
    buffer, one DMA per stripe
  - startup: PE warmup (no-dep matmuls) covers the initial DMA stream;
    inputs arrive in need order (wq per head, xt in 8 pieces) on two
    queues so the first projection can start ~8us in; phase A's last
    norm chain is emitted before the v-projection so the phase B PSUM
    reuse never stalls the PE (stalls trigger HAM half-clock for ~10us)
"""

import os
import sys

sys.path.insert(0, "/opt/trn_rl_repo")

import numpy as np
import ml_dtypes

BF16 = ml_dtypes.bfloat16

B = 2
S = 2048
D = 2048
NH = 16
KVH = 4
HD = 128
THETA = 10000.0
EPS = 1e-6
NHL = NH // KVH  # q heads per core (4)
SCALE = 1.0 / float(np.sqrt(HD))
WARMUP = 32

_CACHED = {}


def build_nc(s=S, d=D, nhl=NHL, hd=HD):
    import concourse.mybir as mybir
    import concourse.tile as tile
    from concourse import bacc
    from concourse.masks import make_identity

    f32 = mybir.dt.float32
    f16 = mybir.dt.float16
    bf16 = mybir.dt.bfloat16
    AF = mybir.ActivationFunctionType

    kc_n = d // 128          # contraction chunks for projections
    nb_n = s // 512          # 512-token blocks
    qt_n = s // 512          # q tiles (512 wide) in attention
    kb_n = s // 128          # k blocks (128 wide)
    db_n = d // 512          # o-proj output chunks

    SHUF = [i ^ 1 for i in range(32)]  # rope partner swap (adjacent pairs)

    nc = bacc.Bacc("TRN2", target_bir_lowering=False, debug=False)

    xT_d = nc.dram_tensor("xT", (d, s), bf16, kind="ExternalInput")
    # weights arrive pre-shuffled by the host into the exact SBUF layout
    # ([partition, kc, m] flattened) so every DMA moves contiguous >=2KB
    # rows; the natural (kc p) m rearrange produced 256B descriptors and a
    # 9us wv transfer
    kc_n_ = d // 128
    wq_d = nc.dram_tensor("wq", (128, kc_n_ * nhl * hd), bf16,
                          kind="ExternalInput")
    wk_d = nc.dram_tensor("wk", (128, kc_n_ * hd), bf16,
                          kind="ExternalInput")
    wv_d = nc.dram_tensor("wv", (128, kc_n_ * hd), bf16,
                          kind="ExternalInput")
    wo_d = nc.dram_tensor("wo", (128, nhl * d), bf16, kind="ExternalInput")
    m1q_d = nc.dram_tensor("m1q", (hd, s), bf16, kind="ExternalInput")
    m2q_d = nc.dram_tensor("m2q", (hd, s), bf16, kind="ExternalInput")
    m1k_d = nc.dram_tensor("m1k", (hd, s), bf16, kind="ExternalInput")
    m2k_d = nc.dram_tensor("m2k", (hd, s), bf16, kind="ExternalInput")
    tri_d = nc.dram_tensor("tri", (128, 128), bf16, kind="ExternalInput")
    y_d = nc.dram_tensor("y", (s, d), f16, kind="ExternalOutput")

    with tile.TileContext(nc) as tc, nc.allow_low_precision(
        reason="bf16 compute by design; fp32 accumulation in PSUM"
    ):
        with (
            tc.tile_pool(name="const", bufs=1) as const,
            tc.tile_pool(name="persist", bufs=1) as persist,
        ):
            # ---- tiny consts first (cheap engine work, no DMA deps) ------
            warm = const.tile([128, 512], bf16, tag="warm")
            nc.vector.memset(warm[:], 0.0)
            ones128 = const.tile([128, 128], bf16, tag="ones128")
            nc.vector.memset(ones128[:], 1.0)
            ident = const.tile([128, 128], bf16, tag="ident")
            make_identity(nc, ident[:])
            tri_sb = const.tile([128, 128], bf16, tag="tri")
            eps_sb = const.tile([128, 1], f32, tag="eps")
            nc.vector.memset(eps_sb[:], EPS)
            dummy = const.tile([1, 1], bf16, tag="dummy")

            # ---- resident weights / coefficients -------------------------
            wq_sb = persist.tile([128, kc_n, nhl * hd], bf16, tag="wq")
            wq_re = wq_d.rearrange("p (kc m) -> p kc m", kc=kc_n)
            wk_sb = persist.tile([128, kc_n, hd], bf16, tag="wk")
            wk_re = wk_d.rearrange("p (kc m) -> p kc m", kc=kc_n)
            wv_sb = persist.tile([128, kc_n, hd], bf16, tag="wv")
            wv_re = wv_d.rearrange("p (kc m) -> p kc m", kc=kc_n)
            wo_sb = persist.tile([128, nhl, d], bf16, tag="wo")
            wo_re = wo_d.rearrange("p (h m) -> p h m", h=nhl)

            m1q_sb = persist.tile([hd, s], bf16, tag="m1q")
            m2q_sb = persist.tile([hd, s], bf16, tag="m2q")
            m1k_sb = persist.tile([hd, s], bf16, tag="m1k")
            m2k_sb = persist.tile([hd, s], bf16, tag="m2k")

            # ---- persistent activations ---------------------------------
            qT_sb = [persist.tile([hd, s], bf16, tag=f"qT{h}", name=f"qT{h}")
                     for h in range(nhl)]
            kT_sb = persist.tile([hd, s], bf16, tag="kT")
            v_sb = persist.tile([128, kb_n, hd + 1], bf16, tag="v")
            nc.vector.memset(v_sb[:, :, hd:hd + 1], 1.0)
            attT_sb = [persist.tile([hd, s], bf16, tag=f"attT{h}",
                                    name=f"attT{h}") for h in range(nhl)]
            # last token block staged persistently: part of its
            # v-projection runs in phase B (as qt0 PE filler), so it must
            # not alias phase B work tiles
            xt3 = persist.tile([128, kc_n, 512], bf16, tag="xt3")

            xT_re = xT_d.rearrange("(kc p) n -> p kc n", p=128)

            # ================= Phase A: projections + norm + rope =========
            with (
                tc.tile_pool(name="xtp", bufs=2) as xtp,
                tc.tile_pool(name="workA", bufs=3) as wa,
                tc.tile_pool(name="psA", bufs=2, space="PSUM") as psA,
            ):
                # PE warmup: dummy matmuls with no DMA deps, so the HAM
                # clock reaches 8/8 and the p-state ramps while the first
                # input DMAs are still in flight.
                wps = psA.tile([128, 512], f32, tag="q_ps", bufs=5,
                               name="wps")
                for _ in range(WARMUP):
                    nc.tensor.matmul(wps[:], warm[:, 0:128], warm[:])

                # ---- input DMAs: need-ordered, two queues ----------------
                # nb0 is consumed kc-ordered (see below), so weights stream
                # kc-major: wk whole (small, needed for every kc group),
                # then wq in 2-kc pieces matching the xt piece arrivals.
                xts = [xtp.tile([128, kc_n, 512], bf16, tag="xt",
                                name="xt0")]
                for j in range(8):   # first block in 8 pieces: matmuls can
                    nc.sync.dma_start(   # start as soon as piece 0 lands
                        xts[0][:, 2 * j:2 * j + 2, :],
                        xT_re[:, 2 * j:2 * j + 2, 0:512])
                nc.gpsimd.dma_start(wk_sb[:], wk_re[:])
                for j in range(8):
                    nc.gpsimd.dma_start(wq_sb[:, 2 * j:2 * j + 2, :],
                                        wq_re[:, 2 * j:2 * j + 2, :])
                nc.gpsimd.dma_start(wv_sb[:], wv_re[:])
                nc.gpsimd.dma_start(m1q_sb[:], m1q_d[:, :])
                nc.gpsimd.dma_start(m2q_sb[:], m2q_d[:, :])
                nc.gpsimd.dma_start(m1k_sb[:], m1k_d[:, :])
                nc.gpsimd.dma_start(m2k_sb[:], m2k_d[:, :])

                def chain_sq(q_ps):
                    sq = wa.tile([128, 512], bf16, tag="sq", name="sq")
                    nc.scalar.activation(sq[:], q_ps[:], AF.Square)
                    return sq

                def chain_rest(sq, q_ps, t, cs):
                    # rmsnorm: broadcast sum-of-squares via one all-ones
                    # matmul; rsqrt = ACT Sqrt + DVE reciprocal (all ACT
                    # funcs stay in the sqrt_and_others table). rope:
                    # partner swap via stream_shuffle, coefficient muls in
                    # bf16 on GPSIMD.
                    ssb = psA.tile([128, 512], f32, tag="ssq", bufs=1,
                                   name="ssb")
                    nc.tensor.matmul(ssb[:], ones128[:], sq[:])
                    srt = wa.tile([128, 512], f32, tag="srt", name="srt")
                    nc.scalar.activation(srt[:], ssb[:], AF.Sqrt,
                                         scale=1.0 / hd, bias=eps_sb[:])
                    rb = wa.tile([128, 512], f32, tag="rb", name="rb")
                    nc.vector.reciprocal_approx_fast(rb[:], srt[:])
                    qn = wa.tile([128, 512], bf16, tag="qn", name="qn")
                    nc.vector.tensor_mul(qn[:], q_ps[:], rb[:])
                    qsw = wa.tile([128, 512], bf16, tag="qsw", name="qsw")
                    nc.vector.stream_shuffle(qsw[:], qn[:], SHUF)
                    m1 = m1q_sb if t < nhl else m1k_sb
                    m2 = m2q_sb if t < nhl else m2k_sb
                    t1 = wa.tile([128, 512], bf16, tag="t1", name="t1")
                    nc.gpsimd.tensor_mul(t1[:], qn[:], m1[:, cs])
                    t2 = wa.tile([128, 512], bf16, tag="t2", name="t2")
                    nc.gpsimd.tensor_mul(t2[:], qsw[:], m2[:, cs])
                    dest = qT_sb[t] if t < nhl else kT_sb
                    nc.gpsimd.tensor_add(dest[:, cs], t1[:], t2[:])

                def norm_rope_chain(q_ps, t, cs):
                    chain_rest(chain_sq(q_ps), q_ps, t, cs)

                pending = None
                for nb in range(nb_n):
                    cs = slice(nb * 512, (nb + 1) * 512)
                    xt = xts[nb]
                    if nb + 1 < nb_n:  # prefetch next block, 1 instruction
                        if nb + 1 == nb_n - 1:
                            nxt = xt3
                        else:
                            nxt = xtp.tile([128, kc_n, 512], bf16,
                                           tag="xt", name=f"xt{nb + 1}")
                        nc.sync.dma_start(
                            nxt[:],
                            xT_re[:, :, (nb + 1) * 512:(nb + 2) * 512])
                        xts.append(nxt)
                    if nb == 1:   # inputs not needed until phase B: keep
                        nc.sync.dma_start(tri_sb[:], tri_d[:, :])
                    if nb == 2:   # early DMA bandwidth for xt prefetches
                        nc.sync.dma_start(wo_sb[:], wo_re[:])

                    if nb == 0:
                        # kc-ordered for the first 12 chunks: 5 concurrent
                        # PSUM accumulators, so each arriving (xt, wq)
                        # piece immediately yields matmuls and the PE
                        # tracks the DMA stream. The last 4 chunks run
                        # t-major so the 5 targets FINISH staggered and
                        # their norm chains don't pile up on ACT at once.
                        q_pss = [psA.tile([128, 512], f32, tag="q_ps",
                                          bufs=5, name=f"q0_{t}")
                                 for t in range(nhl + 1)]
                        for kc in range(kc_n - 4):
                            for t in range(nhl + 1):
                                lhsT = (wq_sb[:, kc, t * hd:(t + 1) * hd]
                                        if t < nhl else wk_sb[:, kc, :])
                                nc.tensor.matmul(
                                    q_pss[t][:], lhsT, xt[:, kc, :],
                                    start=(kc == 0), stop=False,
                                )
                        for t in range(nhl + 1):
                            for kc in range(kc_n - 4, kc_n):
                                lhsT = (wq_sb[:, kc, t * hd:(t + 1) * hd]
                                        if t < nhl else wk_sb[:, kc, :])
                                nc.tensor.matmul(
                                    q_pss[t][:], lhsT, xt[:, kc, :],
                                    start=False, stop=(kc == kc_n - 1),
                                )
                            if pending is not None:
                                norm_rope_chain(*pending)
                            pending = (q_pss[t], t, cs)
                    else:
                        # q heads then k (k FIRST in the last block, so
                        # phase B's first S matmul never waits on the k
                        # chain): projection MMs now, chain deferred one
                        # tensor so its ACT/DVE/GPS work overlaps the next
                        # target's projection matmuls
                        t_order = ([nhl] + list(range(nhl))
                                   if nb == nb_n - 1 else
                                   list(range(nhl + 1)))
                        tail2 = []
                        for idx, t in enumerate(t_order):
                            q_ps = psA.tile([128, 512], f32, tag="q_ps",
                                            bufs=5)
                            for kc in range(kc_n):
                                if t < nhl:
                                    lhsT = wq_sb[:, kc, t * hd:(t + 1) * hd]
                                else:
                                    lhsT = wk_sb[:, kc, :]
                                nc.tensor.matmul(
                                    q_ps[:], lhsT, xt[:, kc, :],
                                    start=(kc == 0), stop=(kc == kc_n - 1),
                                )
                            if nb == nb_n - 1 and idx >= 3:
                                # last block's last two chains are SPLIT:
                                # ACT Square now, the rest emitted between
                                # v-proj fills below -- the in-order PE
                                # queue must never reach a chain matmul
                                # before its ACT input is ready
                                if pending is not None:
                                    norm_rope_chain(*pending)
                                    pending = None
                                tail2.append((chain_sq(q_ps), q_ps, t))
                            else:
                                if pending is not None:
                                    norm_rope_chain(*pending)
                                pending = (q_ps, t, cs)

                    if nb == nb_n - 1:
                        # interleave the last block's v-projection (pure PE
                        # work) with the split chain remainders, so the
                        # chains' ACT/DVE/GPS latency is fully hidden and
                        # phase B starts with all PSUM banks ready
                        def vproj3(tb):
                            vp = psA.tile([128, 512], f32, tag="vps",
                                          bufs=2, name="v_ps3")
                            for kc in range(kc_n):
                                nc.tensor.matmul(
                                    vp[:, 0:hd],
                                    xt[:, kc, tb * 128:(tb + 1) * 128],
                                    wv_sb[:, kc, :],
                                    start=(kc == 0), stop=(kc == kc_n - 1),
                                )
                            nc.scalar.copy(
                                v_sb[:, 12 + tb, 0:hd], vp[:, 0:hd])
                        vproj3(0)
                        chain_rest(*tail2[0], cs)
                        vproj3(1)
                        chain_rest(*tail2[1], cs)
                        vproj3(2)
                        # boundary bridge: no-dep matmuls keep the PE busy
                        # (and HAM at 8/8) while the last chains' DVE work
                        # frees the PSUM banks phase B reuses
                        brg = psA.tile([128, 512], f32, tag="vps", bufs=2,
                                       name="bridge")
                        for _ in range(8):
                            nc.tensor.matmul(brg[:], warm[:, 0:128],
                                             warm[:])
                        continue

                    # v: token-major directly (lhsT = x block, rhs = wv
                    # chunk) -> no transposes
                    v_ps = psA.tile([128, 512], f32, tag="vps", bufs=2,
                                    name="v_ps")
                    for tb in range(4):
                        for kc in range(kc_n):
                            nc.tensor.matmul(
                                v_ps[:, tb * 128:(tb + 1) * 128],
                                xt[:, kc, tb * 128:(tb + 1) * 128],
                                wv_sb[:, kc, :],
                                start=(kc == 0), stop=(kc == kc_n - 1),
                            )
                    for tb in range(4):
                        nc.vector.tensor_copy(
                            v_sb[:, nb * 4 + tb, 0:hd],
                            v_ps[:, tb * 128:(tb + 1) * 128])
                if pending is not None:
                    norm_rope_chain(*pending)
                    pending = None

            # ================= Phase B: causal flash attention ============
            with (
                tc.tile_pool(name="workB", bufs=4) as wb,
                tc.tile_pool(name="psB", bufs=2, space="PSUM") as psB,
            ):
                backlog = []   # (tt, db) o-proj units, drained in kb loops
                ybig = {}

                # preload the Exp ACT table during phase A's tail (the
                # table load binds to the first Exp in ACT queue order);
                # the dummy tile lives in the const pool so the exp never
                # waits on recycled work-tile memory
                nc.scalar.activation(dummy[:], eps_sb[0:1, 0:1], AF.Exp)

                vleft = [3]   # last nb3 v-proj block: qt0 PE filler

                def vproj_b(tb):
                    vp = psB.tile([128, hd + 1], f32, tag="att", bufs=4,
                                  name="vps_b")
                    for kc in range(kc_n):
                        nc.tensor.matmul(
                            vp[:, 0:hd],
                            xt3[:, kc, tb * 128:(tb + 1) * 128],
                            wv_sb[:, kc, :],
                            start=(kc == 0), stop=(kc == kc_n - 1),
                        )
                    nc.vector.tensor_copy(v_sb[:, 12 + tb, 0:hd],
                                          vp[:, 0:hd])

                def oproj_unit(tt, db, act_cast=False):
                    if db == 0:
                        ybig[tt] = wb.tile([128, d], f16, tag="ybig",
                                           bufs=2, name=f"ybig{tt}")
                    y_ps = psB.tile([128, 1024], f32, tag="spair", bufs=2,
                                    name="y_ps")
                    for hh in range(nhl):
                        nc.tensor.matmul(
                            y_ps[:, 0:512],
                            attT_sb[hh][:, tt * 128:(tt + 1) * 128],
                            wo_sb[:, hh, db * 512:(db + 1) * 512],
                            start=(hh == 0), stop=(hh == nhl - 1),
                        )
                    dst = ybig[tt][:, db * 512:(db + 1) * 512]
                    if act_cast:
                        nc.scalar.copy(dst, y_ps[:, 0:512])
                    else:
                        nc.vector.tensor_copy(dst, y_ps[:, 0:512])
                    if db == db_n - 1:
                        nc.sync.dma_start(
                            y_d[tt * 128:(tt + 1) * 128, :], ybig[tt][:])
                        del ybig[tt]

                prev_tp = None   # previous head's normalized stripes:
                                 # its transposes run under the NEXT
                                 # head's S matmuls (the per-head tail is
                                 # inherently serial otherwise)

                def flush_tp():
                    ph, pqt, patt_n = prev_tp
                    for qs in range(4):
                        # transpose target shares the att tag (bitcast
                        # bf16 view) to stay within 8 PSUM banks
                        tpt = psB.tile([128, hd + 1], f32, tag="att",
                                       bufs=4, name="tp")
                        tp = tpt[:, 0:64].bitcast(bf16)
                        nc.tensor.transpose(tp, patt_n[qs][:], ident[:])
                        nc.vector.tensor_copy(
                            attT_sb[ph][:, pqt * 512 + qs * 128:
                                        pqt * 512 + (qs + 1) * 128],
                            tp,
                        )

                for qt in range(qt_n):
                    # spread the available o-proj units evenly over this
                    # qt group's pairs (clustered draining leaves the
                    # remaining pairs ACT-bound by ~200ns each)
                    qt_pairs = nhl * (2 * qt + 2)
                    qt_backlog0 = len(backlog)
                    qt_pc = 0
                    qt_drained = 0
                    for h in range(nhl):
                        nkb = 4 * qt + 4
                        np_ = nkb // 2   # k-block pairs
                        att_n = {}
                        s_tiles = {}

                        def emit_pair(j):
                            # two k-blocks' scores into one 2-bank PSUM
                            # tile (each matmul stays within one bank).
                            # Diagonal blocks get the causal mask ADDED in
                            # PSUM (-6e4 above the diagonal, via mask.T @
                            # identity): exp then yields exact zeros, so
                            # no per-block mask multiply serializes the
                            # exp->PV chain
                            sp = psB.tile([128, 1024], f32, tag="spair",
                                          name="s_ps", bufs=2)
                            for i in range(2):
                                kb = 2 * j + i
                                r = kb - 4 * qt
                                c0 = 128 * r if r > 0 else 0
                                nc.tensor.matmul(
                                    sp[:, 512 * i + c0:512 * (i + 1)],
                                    kT_sb[:, kb * 128:(kb + 1) * 128],
                                    qT_sb[h][:,
                                             qt * 512 + c0:(qt + 1) * 512],
                                )
                                if r >= 0:
                                    nc.tensor.matmul(
                                        sp[:, 512 * i + c0:
                                           512 * i + c0 + 128],
                                        tri_sb[:], ident[:],
                                        start=False, stop=True,
                                        skip_group_check=True,
                                    )
                            s_tiles[j] = sp

                        emit_pair(0)
                        if np_ > 1:
                            emit_pair(1)
                        if prev_tp is not None:
                            flush_tp()
                        att = [psB.tile([128, hd + 1], f32, tag="att",
                                        bufs=4, name=f"att{i}")
                               for i in range(4)]
                        for j in range(np_):
                            if j + 2 < np_:
                                emit_pair(j + 2)
                            # PE filler while ACT computes exp: o-proj
                            # units (proportionally spread), or the
                            # deferred v-projection during qt=0
                            qt_pc += 1
                            target = (qt_pc * qt_backlog0 +
                                      qt_pairs - 1) // qt_pairs
                            while backlog and qt_drained < target:
                                oproj_unit(*backlog.pop(0))
                                qt_drained += 1
                            if qt == 0 and vleft and j == 0:
                                vproj_b(vleft.pop(0))
                            sp = s_tiles.pop(j)
                            p = wb.tile([128, 1024], bf16, tag="p", bufs=4)
                            kb0 = 2 * j
                            r0 = kb0 - 4 * qt
                            if r0 + 1 < 0:
                                nc.scalar.activation(p[:], sp[:], AF.Exp,
                                                     scale=SCALE)
                            else:
                                # diagonal pair: exp the two valid regions,
                                # tri-mask the diagonal blocks on GPSIMD
                                c00 = 128 * r0 if r0 > 0 else 0
                                c01 = 128 * (r0 + 1)
                                nc.scalar.activation(
                                    p[:, c00:512], sp[:, c00:512],
                                    AF.Exp, scale=SCALE)
                                nc.scalar.activation(
                                    p[:, 512 + c01:1024],
                                    sp[:, 512 + c01:1024],
                                    AF.Exp, scale=SCALE)
                            for i in range(2):
                                kb = 2 * j + i
                                off = 512 * i
                                for qs in range(4):
                                    kmax = 4 * qt + qs
                                    if kb > kmax:
                                        continue
                                    nc.tensor.matmul(
                                        att[qs][:],
                                        p[:, off + qs * 128:
                                          off + (qs + 1) * 128],
                                        v_sb[:, kb, :],
                                        start=(kb == 0), stop=(kb == kmax),
                                    )
                                # the DVE normalize of a finished q stripe
                                # starts NOW, overlapping the remaining
                                # pairs; only the PE transposes stay in
                                # the head's tail
                                r = kb - 4 * qt
                                if r >= 0:
                                    rec = wb.tile([128, 1], f32, tag="rec",
                                                  bufs=4, name="rec")
                                    nc.vector.reciprocal(
                                        rec[:], att[r][:, hd:hd + 1])
                                    att_n[r] = wb.tile([128, 128], bf16,
                                                       tag="attn", bufs=4,
                                                       name=f"attn{r}")
                                    nc.vector.tensor_scalar_mul(
                                        att_n[r][:], att[r][:, 0:hd],
                                        rec[:])
                        prev_tp = (h, qt, att_n)
                    for tt in range(qt * 4, qt * 4 + 4):
                        for db in range(db_n):
                            backlog.append((tt, db))
                if prev_tp is not None:
                    flush_tp()
                    prev_tp = None
                n_tail = 0
                while backlog:
                    oproj_unit(*backlog.pop(0), act_cast=bool(n_tail % 2))
                    n_tail += 1

    nc.compile()
    return nc


def _rope_coeffs(norm_w, s=S, hd=HD):
    """Coefficient tiles [hd, s]: interleaved rope pairing (partner =
    adjacent lane), norm weight folded in.
      dest[2i]   = qn[2i]*cos_i*w[2i]   + qn[2i+1]*(-sin_i*w[2i])
      dest[2i+1] = qn[2i+1]*cos_i*w[2i+1] + qn[2i]*( sin_i*w[2i+1])
    with qs = shuffle(qn, pair swap), dest = qn*m1 + qs*m2."""
    w = np.asarray(norm_w, np.float64)
    pos = np.arange(s, dtype=np.float64)
    inv_freq = 1.0 / (THETA ** (np.arange(0, hd, 2, dtype=np.float64) / hd))
    ang = pos[None, :] * inv_freq[:, None]          # [half, s]
    cos, sin = np.cos(ang), np.sin(ang)
    m1 = np.empty((hd, s), np.float32)
    m2 = np.empty((hd, s), np.float32)
    m1[0::2] = cos * w[0::2, None]
    m1[1::2] = cos * w[1::2, None]
    m2[0::2] = -sin * w[0::2, None]
    m2[1::2] = sin * w[1::2, None]
    return m1, m2


def _host_prep(x, wq, wk, wv, wo, q_norm_w, k_norm_w):
    m1q, m2q = _rope_coeffs(q_norm_w)
    m1k, m2k = _rope_coeffs(k_norm_w)
    # additive causal mask, transposed for lhsT (mask.T @ I = mask):
    # M[k, q] = -6e4 where k > q; lhsT = M.T
    m = np.where(np.arange(128)[:, None] > np.arange(128)[None, :],
                 np.float32(-60000.0), np.float32(0.0))
    tri = np.ascontiguousarray(m.T).astype(BF16)

    def shuf_w(w):
        # (kc*128, m) -> (128, kc*m): row p holds [kc, m] in SBUF order
        kc = w.shape[0] // 128
        return np.ascontiguousarray(
            w.reshape(kc, 128, w.shape[1]).transpose(1, 0, 2)
            .reshape(128, kc * w.shape[1]))

    in_maps = []
    for c in range(8):
        b, g = c // 4, c % 4
        in_maps.append({
            "xT": np.ascontiguousarray(x[b].T).astype(BF16),
            "wq": shuf_w(
                wq[:, NHL * g * HD:NHL * (g + 1) * HD]).astype(BF16),
            "wk": shuf_w(wk[:, g * HD:(g + 1) * HD]).astype(BF16),
            "wv": shuf_w(wv[:, g * HD:(g + 1) * HD]).astype(BF16),
            "wo": shuf_w(
                wo[NHL * g * HD:NHL * (g + 1) * HD, :]).astype(BF16),
            "m1q": m1q.astype(BF16), "m2q": m2q.astype(BF16),
            "m1k": m1k.astype(BF16), "m2k": m2k.astype(BF16),
            "tri": tri,
        })
    return in_maps


def _install_ntff_shim():
    import types
    if "antenv.axon_hooks" in sys.modules:
        return
    mod = types.ModuleType("antenv.axon_hooks")
    _hook = [None]
    mod.set_axon_ntff_profile_hook = lambda h: _hook.__setitem__(0, h)
    mod.get_axon_ntff_profile_hook = lambda: _hook[0]
    sys.modules["antenv.axon_hooks"] = mod
    try:
        from trn_agent_boot.trn_boot import _ntff_profile_via_ctypes
        mod.set_axon_ntff_profile_hook(
            _ntff_profile_via_ctypes("/opt/axon/libaxon_pjrt.so")
        )
    except Exception:
        pass


LAST_EXEC_NS = None


def kernel(x, wq, wk, wv, wo, q_norm_w, k_norm_w):
    global LAST_EXEC_NS
    from concourse import bass_utils

    x = np.asarray(x)
    if "nc" not in _CACHED:
        _CACHED["nc"] = build_nc()
    nc = _CACHED["nc"]

    in_maps = _host_prep(
        np.asarray(x, np.float32), np.asarray(wq, np.float32),
        np.asarray(wk, np.float32), np.asarray(wv, np.float32),
        np.asarray(wo, np.float32), np.asarray(q_norm_w, np.float32),
        np.asarray(k_norm_w, np.float32),
    )
    trace = bool(int(os.environ.get("BASS_KERNEL_TRACE", "0")))
    if trace:
        _install_ntff_shim()
    res = bass_utils.run_bass_kernel_spmd(
        nc, in_maps, core_ids=list(range(8)), trace=trace
    )
    LAST_EXEC_NS = res.exec_time_ns
    y = np.zeros((B, S, D), np.float32)
    for c in range(8):
        y[c // 4] += res.results[c]["y"]
    return y
